# revision 34
# baseline (speedup 1.0000x reference)
"""Trainium2 Bass kernel for nn_DCAM (dense transformer attention module).

Reference computation (per batch b):
  qp/kp/vp = avg_pool2d(feature_{q,k,v}, 2)            # (C=256, 64, 64)
  q = Wq @ qp, k = Wk @ kp  (M=32 channels)            # (32, N=4096)
  v = Wv @ vp                                          # (256, N)
  attn = softmax(q^T k, axis=-1)                       # (N, N)
  out[c, m] = sum_n v[c, n] attn[m, n]                 # (256, N)
  result = upsample_nearest(out, 2) + feature_v        # (256, 128, 128)

Sharding: data-parallel over batch B=8 across 8 NeuronCores (1 batch/core).

Per-core design (v5 — fp8 DoubleRow attention + streamed phase A):
  - p = exp(s/16 - 5) written by ACT directly as fp8e5m2.
  - out-matmuls fp8 DoubleRow (2 j-blocks per MM): lhsT = vt e4m3
    [128,2,128], rhs = p e5m2 [128,2,512].
  - denominator: DoubleRow ones(=16.0) matmul -> l replicated on all 128
    partitions; reciprocal_approx_fast directly on PSUM, no DRAM bounce.
  - v rotated by Hadamard R=H_256/16 folded into Wv on host; un-rotated
    by 4 bf16 matmuls per i-chunk, deferred 2 steps into the next chunk
    so PE never head-blocks on the DVE normalize chain.
  - phase A: fk/fv/fq loaded as RAW contiguous rows per 16-row chunk;
    pooling = one contiguous dy-add on DVE + dx-shift pairs folded into
    the projection matmuls (2 accumulating MMs per (cb,dx)). No strided
    GpSimd adds. i-chunk-0 attention steps are emitted per input chunk,
    so EXP/PE start as soon as chunk 0 has landed.
  - output written bf16 (host upcasts).
"""
import numpy as np
import ml_dtypes

import concourse.bass as bass
import concourse.mybir as mybir
import concourse.tile as tile
from concourse import bacc
from concourse.bass_utils import run_bass_kernel_spmd

F32 = mybir.dt.float32
BF16 = mybir.dt.bfloat16
FP8E4 = mybir.dt.float8e4
FP8E5 = mybir.dt.float8e5
AF = mybir.ActivationFunctionType
DR = mybir.MatmulPerfMode.DoubleRow

B = 8
C = 256
M = 32
H = W = 128
HP = WP = 64
N = HP * WP          # 4096
CB = C // 128        # 2 channel blocks
JB = N // 128        # 32 key blocks
JG = JB // 4         # 8 groups of 4 packed j-blocks
IC = N // 512        # 8 query chunks

EXP_BIAS = -5.0
NSTEP = 16


def build_module():
    nc = bacc.Bacc("TRN2", target_bir_lowering=False, debug=False)

    fq_d = nc.dram_tensor("feature_q", [C, H, W], BF16, kind="ExternalInput").ap()
    fk_d = nc.dram_tensor("feature_k", [C, H, W], BF16, kind="ExternalInput").ap()
    fv_d = nc.dram_tensor("feature_v", [C, H, W], BF16, kind="ExternalInput").ap()
    wq_d = nc.dram_tensor("WqT", [C, M], BF16, kind="ExternalInput").ap()
    wk_d = nc.dram_tensor("WkT", [C, M], BF16, kind="ExternalInput").ap()
    wv_d = nc.dram_tensor("WvT", [C, C], BF16, kind="ExternalInput").ap()
    r_d = nc.dram_tensor("Rmat", [C, C], BF16, kind="ExternalInput").ap()
    out_d = nc.dram_tensor("out", [C, H, W], BF16, kind="ExternalOutput").ap()

    with tile.TileContext(nc) as tc:
        with tc.tile_pool(name="const", bufs=1) as cpool, \
             tc.tile_pool(name="persist", bufs=1) as pp, \
             tc.tile_pool(name="ps", bufs=1, space="PSUM") as ps, \
             tc.tile_pool(name="wk", bufs=1) as wkp, \
             tc.tile_pool(name="poolB", bufs=1) as pb:
            # ---- constants ----
            wq_sb = cpool.tile([128, CB, M], BF16, name="wq_sb")
            nc.sync.dma_start(wq_sb[:], wq_d.rearrange("(b p) m -> p b m", p=128))
            wk_sb = cpool.tile([128, CB, M], BF16, name="wk_sb")
            nc.sync.dma_start(wk_sb[:], wk_d.rearrange("(b p) m -> p b m", p=128))
            wv_sb = cpool.tile([128, CB, C], BF16, name="wv_sb")
            nc.sync.dma_start(wv_sb[:], wv_d.rearrange("(b p) c -> p b c", p=128))
            r_sb = cpool.tile([128, CB, C], BF16, name="r_sb")
            nc.sync.dma_start(r_sb[:], r_d.rearrange("(b p) c -> p b c", p=128))
            ones16 = cpool.tile([128, 2, 128], FP8E4, name="ones16")
            nc.vector.memset(ones16, 16.0)
            bias_t = cpool.tile([128, 1], F32, name="bias_t")
            nc.vector.memset(bias_t, EXP_BIAS)

            # ---- persistent tensors ----
            q4 = pp.tile([128, N], BF16, name="q4")          # q replicated x4
            kh = pp.tile([128, JG, 128], BF16, name="kh")    # [32*t+m, jg, jf]
            vt = pp.tile([128, JB, C], FP8E4, name="vt")     # vT[j, c] per jb
            fv_sb = pp.tile([128, CB, H, W], BF16, name="fv_sb")

            # ---- streamed projection of one 16-raw-row chunk.
            # q_fetch (DMA + pooling dy-add) is emitted ~9 steps before
            # q_proj's matmuls so the PE FIFO never head-blocks on the
            # fq DMA chain.
            qd_q = {}

            def q_fetch(icn, gps_dy):
                qr = wkp.tile([128, CB, 16, W], BF16, tag="qr", bufs=2,
                              name="qr")
                for cb in range(CB):
                    nc.gpsimd.dma_start(
                        qr[:, cb],
                        fq_d[cb * 128:(cb + 1) * 128,
                             icn * 16:(icn + 1) * 16, :])
                qd = wkp.tile([128, CB, 8, W], BF16, tag="qd", bufs=2,
                              name="qd")
                for cb in range(CB):
                    src = qr[:, cb].rearrange("c (h dy) w -> c h dy w", dy=2)
                    eng = nc.gpsimd if gps_dy else nc.vector
                    eng.tensor_add(qd[:, cb], src[:, :, 0], src[:, :, 1])
                qd_q[icn] = qd

            def q_proj(icn):
                qd = qd_q.pop(icn)
                pr = ps.tile([32, 512], F32, tag="u", bufs=1, name="prq")
                i = 0
                for cb in range(CB):
                    qdx = qd[:, cb].rearrange("c h (w dx) -> c h w dx", dx=2)
                    for dx in range(2):
                        nc.tensor.matmul(pr[:], wq_sb[:, cb],
                                         qdx[:, :, :, dx],
                                         start=(i == 0), stop=(i == 3),
                                         skip_group_check=True)
                        i += 1
                cs = slice(icn * 512, (icn + 1) * 512)
                nc.vector.tensor_copy(q4[0:32, cs], pr[:])
                for g in range(1, 4):
                    nc.sync.dma_start(q4[g * 32:(g + 1) * 32, cs],
                                      q4[0:32, cs])

            def q_pipeline(icn, gps_dy):
                q_fetch(icn, gps_dy)
                q_proj(icn)

            def kv_chunk(c):
                # fv chunk: raw rows into persistent fv_sb (residual+proj)
                for cb in range(CB):
                    nc.gpsimd.dma_start(
                        fv_sb[:, cb, c * 16:(c + 1) * 16, :],
                        fv_d[cb * 128:(cb + 1) * 128,
                             c * 16:(c + 1) * 16, :])
                # fk chunk: raw rows, transient
                fkr = wkp.tile([128, CB, 16, W], BF16, tag="fkr", bufs=3,
                               name="fkr")
                for cb in range(CB):
                    eng = nc.sync if cb == 0 else nc.scalar
                    eng.dma_start(
                        fkr[:, cb],
                        fk_d[cb * 128:(cb + 1) * 128,
                             c * 16:(c + 1) * 16, :])
                # dy-adds (contiguous, DVE)
                kd = wkp.tile([128, CB, 8, W], BF16, tag="kd", bufs=2,
                              name="kd")
                for cb in range(CB):
                    src = fkr[:, cb].rearrange("c (h dy) w -> c h dy w", dy=2)
                    nc.vector.tensor_add(kd[:, cb], src[:, :, 0],
                                         src[:, :, 1])
                vd = wkp.tile([128, CB, 8, W], BF16, tag="vd", bufs=2,
                              name="vd")
                for cb in range(CB):
                    src = fv_sb[:, cb, c * 16:(c + 1) * 16, :].rearrange(
                        "c (h dy) w -> c h dy w", dy=2)
                    nc.gpsimd.tensor_add(vd[:, cb], src[:, :, 0],
                                         src[:, :, 1])
                # k-proj: dx pairs folded as accumulating matmuls
                pr = ps.tile([32, 512], F32, tag="u", bufs=1, name="prk")
                i = 0
                for cb in range(CB):
                    kdx = kd[:, cb].rearrange("c h (w dx) -> c h w dx", dx=2)
                    for dx in range(2):
                        nc.tensor.matmul(pr[:], wk_sb[:, cb],
                                         kdx[:, :, :, dx],
                                         start=(i == 0), stop=(i == 3),
                                         skip_group_check=True)
                        i += 1
                for t in range(4):
                    nc.vector.tensor_copy(kh[t * 32:(t + 1) * 32, c, :],
                                          pr[:, t * 128:(t + 1) * 128])
                # v-proj: per j-block (4 per chunk)
                for r in range(4):
                    jb = 4 * c + r
                    vt_ps = ps.tile([128, 512], F32, tag="u", bufs=1,
                                    name="vt_ps")[:, :C]
                    i = 0
                    for cb in range(CB):
                        vdx = vd[:, cb, 2 * r:2 * r + 2, :].rearrange(
                            "c h (w dx) -> c h w dx", dx=2)
                        for dx in range(2):
                            nc.tensor.matmul(vt_ps,
                                             vdx[:, :, :, dx],
                                             wv_sb[:, cb],
                                             start=(i == 0), stop=(i == 3),
                                             skip_group_check=True)
                            i += 1
                    nc.scalar.copy(vt[:, jb, :], vt_ps[:])

            # ---- phase B machinery ----
            def emit_S(ic, st, s_tile):
                i0 = ic * 512
                jgrp = st // 2
                for u in range(2):
                    t = 2 * (st % 2) + u
                    gs = slice(t * 32, (t + 1) * 32)
                    nc.tensor.matmul(s_tile[:, u, :], kh[gs, jgrp, :],
                                     q4[gs, i0:i0 + 512],
                                     start=True, stop=True,
                                     tile_position=(t * 32, 0),
                                     skip_group_check=True)

            def new_s():
                return ps.tile([128, 2, 512], F32, tag="s", bufs=2,
                               name="s_t")

            def epilogue_pe(ic, oc):
                # un-rotate: out2[c, i] = sum_c' R[c', c] oc[c', i]
                for cb in range(CB):
                    u_ps = ps.tile([128, 512], F32, tag="u", bufs=1,
                                   name="u_ps")
                    for cbp in range(CB):
                        nc.tensor.matmul(
                            u_ps[:], r_sb[:, cbp, cb * 128:(cb + 1) * 128],
                            oc[cbp][:],
                            start=(cbp == 0), stop=(cbp == CB - 1),
                            skip_group_check=True)
                    out3 = pb.tile([128, 512], BF16, tag="out3", bufs=2,
                                   name="out3")
                    nc.vector.tensor_copy(out3[:], u_ps[:])
                    final = pb.tile([128, 8, 2, WP, 2], BF16,
                                    tag="final", bufs=2, name="final")
                    up = out3.rearrange("c (h w) -> c h w", w=WP)[
                        :, :, :, None].to_broadcast((128, 8, WP, 2))
                    fvv = fv_sb[:, cb, ic * 16:(ic + 1) * 16, :].rearrange(
                        "c (h dy) (w dx) -> c h dy w dx", dy=2, dx=2)
                    nc.vector.tensor_add(final[:, :, 0], up, fvv[:, :, 0])
                    nc.gpsimd.tensor_add(final[:, :, 1], up, fvv[:, :, 1])
                    nc.sync.dma_start(
                        out_d[cb * 128:(cb + 1) * 128,
                              ic * 16:(ic + 1) * 16, :],
                        final.rearrange("c h dy w dx -> c (h dy) (w dx)"))

            # ---- interleaved schedule ----
            state = {"o": None, "l": None}
            s_q = []
            pending = []

            def begin_ic(ic):
                state["o"] = [ps.tile([128, 512], F32, tag=f"o{cb}",
                                      bufs=1, name=f"o{cb}_ps")
                              for cb in range(CB)]
                state["l"] = ps.tile([128, 512], F32, tag="l", bufs=1,
                                     name="l_ps")

            def run_step(ic, st):
                s_cur = s_q.pop(0)
                nxt = (ic, st + 2) if st + 2 < NSTEP else (
                    (ic + 1, st - 14) if ic + 1 < IC else None)
                if nxt is not None:
                    s_new = new_s()
                    emit_S(nxt[0], nxt[1], s_new)
                    s_q.append(s_new)
                p = pb.tile([128, 2, 512], FP8E5, tag="p", bufs=4, name="p")
                nc.scalar.activation(p[:], s_cur[:], AF.Exp,
                                     scale=0.0625, bias=bias_t[:])
                nc.tensor.matmul(
                    state["l"][:], ones16[:], p[:],
                    start=(st == 0), stop=(st == NSTEP - 1),
                    perf_mode=DR, skip_group_check=True)
                for cb in range(CB):
                    nc.tensor.matmul(
                        state["o"][cb][:],
                        vt[:, 2 * st:2 * st + 2, cb * 128:(cb + 1) * 128],
                        p[:],
                        start=(st == 0), stop=(st == NSTEP - 1),
                        perf_mode=DR, skip_group_check=True)
                if st == 5 and pending:
                    pending.pop(0)()
                if st == 2 and ic + 1 < IC:
                    q_proj(ic + 1)
                if st == 11 and ic + 2 < IC:
                    q_fetch(ic + 2, gps_dy=True)

            def end_ic(ic):
                # epilogue head: free PSUM banks fast
                rb = pb.tile([128, 512], F32, tag="rb", bufs=2, name="rb")
                nc.vector.reciprocal_approx_fast(out=rb[:],
                                                 in_=state["l"][:])
                oc = []
                for cb in range(CB):
                    osb = pb.tile([128, 512], BF16, tag=f"osb{cb}", bufs=2,
                                  name=f"osb{cb}")
                    nc.vector.tensor_copy(osb[:], state["o"][cb][:])
                    t_ = pb.tile([128, 512], BF16, tag=f"oc{cb}", bufs=2,
                                 name=f"oc{cb}")
                    nc.vector.tensor_mul(t_[:], osb[:], rb[:])
                    oc.append(t_)
                pending.append(lambda ic_=ic, oc_=oc: epilogue_pe(ic_, oc_))

            # phase A': stream chunks; interleave ic0 steps with 1-chunk lag
            q_pipeline(0, gps_dy=False)
            begin_ic(0)
            for c in range(IC):
                kv_chunk(c)
                if c == 1:
                    q_fetch(1, gps_dy=True)
                if c == 0:
                    s_q.append(new_s())
                    emit_S(0, 0, s_q[0])
                    s_q.append(new_s())
                    emit_S(0, 1, s_q[1])
                else:
                    run_step(0, 2 * (c - 1))
                    run_step(0, 2 * (c - 1) + 1)
            run_step(0, 14)
            run_step(0, 15)
            end_ic(0)
            # steady state: ic 1..7
            for ic in range(1, IC):
                begin_ic(ic)
                for st in range(NSTEP):
                    run_step(ic, st)
                end_ic(ic)
            for fn in pending:
                fn()

    nc.compile()
    return nc


def _hadamard(n):
    h = np.array([[1.0]], dtype=np.float64)
    while h.shape[0] < n:
        h = np.block([[h, h], [h, -h]])
    return h / np.sqrt(n)


_NC_CACHE = []
LAST_RESULT = []  # last BassKernelResults, for perf inspection by test.py


def kernel(**inputs) -> np.ndarray:
    bf = ml_dtypes.bfloat16
    fq = np.ascontiguousarray(np.asarray(inputs["feature_q"]).astype(bf))
    fk = np.ascontiguousarray(np.asarray(inputs["feature_k"]).astype(bf))
    fv = np.ascontiguousarray(np.asarray(inputs["feature_v"]).astype(bf))
    wq = np.asarray(inputs["Wq"], dtype=np.float32)
    wk = np.asarray(inputs["Wk"], dtype=np.float32)
    wv = np.asarray(inputs["Wv"], dtype=np.float32)

    # on-device pooling is a 2x2 *sum*; q,k each pick up 4x -> s is 16x,
    # folded into the exp scale. v picks up 4x from pooling; another 4x
    # here scales v into fp8e4 range (16x total, cancelled by the
    # 16.0-weighted denominator). Wv is pre-rotated by R (Hadamard/16,
    # orthonormal, symmetric, bf16-exact); the kernel un-rotates after
    # the attention average.
    R = _hadamard(256)
    wqt = np.ascontiguousarray(wq.T.astype(bf))               # (C, M)
    wkt = np.ascontiguousarray(wk.T.astype(bf))
    wvt = np.ascontiguousarray(((R @ wv).T * 4.0).astype(bf))  # (C, C)
    rmat = np.ascontiguousarray(R.astype(bf))                  # (C, C)

    if not _NC_CACHE:
        _NC_CACHE.append(build_module())
    nc = _NC_CACHE[0]

    in_maps = [
        {
            "feature_q": fq[b],
            "feature_k": fk[b],
            "feature_v": fv[b],
            "WqT": wqt,
            "WkT": wkt,
            "WvT": wvt,
            "Rmat": rmat,
        }
        for b in range(B)
    ]
    res = run_bass_kernel_spmd(nc, in_maps, core_ids=list(range(B)))
    LAST_RESULT.clear()
    LAST_RESULT.append(res)
    out = np.stack([res.results[b]["out"] for b in range(B)], axis=0)
    return out.astype(np.float32)


if __name__ == "__main__":
    nc = build_module()
    print("module built + compiled OK")


# revision 35
# speedup vs baseline: 1.0402x; 1.0402x over previous
"""Trainium2 Bass kernel for nn_DCAM (dense transformer attention module).

Reference computation (per batch b):
  qp/kp/vp = avg_pool2d(feature_{q,k,v}, 2)            # (C=256, 64, 64)
  q = Wq @ qp, k = Wk @ kp  (M=32 channels)            # (32, N=4096)
  v = Wv @ vp                                          # (256, N)
  attn = softmax(q^T k, axis=-1)                       # (N, N)
  out[c, m] = sum_n v[c, n] attn[m, n]                 # (256, N)
  result = upsample_nearest(out, 2) + feature_v        # (256, 128, 128)

Sharding: data-parallel over batch B=8 across 8 NeuronCores (1 batch/core).

Per-core design (v5 — fp8 DoubleRow attention + streamed phase A):
  - p = exp(s/16 - 5) written by ACT directly as fp8e5m2.
  - out-matmuls fp8 DoubleRow (2 j-blocks per MM): lhsT = vt e4m3
    [128,2,128], rhs = p e5m2 [128,2,512].
  - denominator: DoubleRow ones(=16.0) matmul -> l replicated on all 128
    partitions; reciprocal_approx_fast directly on PSUM, no DRAM bounce.
  - v rotated by Hadamard R=H_256/16 folded into Wv on host; un-rotated
    by 4 bf16 matmuls per i-chunk, deferred 2 steps into the next chunk
    so PE never head-blocks on the DVE normalize chain.
  - phase A: fk/fv/fq loaded as RAW contiguous rows per 16-row chunk;
    pooling = one contiguous dy-add on DVE + dx-shift pairs folded into
    the projection matmuls (2 accumulating MMs per (cb,dx)). No strided
    GpSimd adds. i-chunk-0 attention steps are emitted per input chunk,
    so EXP/PE start as soon as chunk 0 has landed.
  - output written bf16 (host upcasts).
"""
import numpy as np
import ml_dtypes

import concourse.bass as bass
import concourse.mybir as mybir
import concourse.tile as tile
from concourse import bacc
from concourse.bass_utils import run_bass_kernel_spmd

F32 = mybir.dt.float32
BF16 = mybir.dt.bfloat16
FP8E4 = mybir.dt.float8e4
FP8E5 = mybir.dt.float8e5
AF = mybir.ActivationFunctionType
DR = mybir.MatmulPerfMode.DoubleRow

B = 8
C = 256
M = 32
H = W = 128
HP = WP = 64
N = HP * WP          # 4096
CB = C // 128        # 2 channel blocks
JB = N // 128        # 32 key blocks
JG = JB // 4         # 8 groups of 4 packed j-blocks
IC = N // 512        # 8 query chunks

EXP_BIAS = -5.0
NSTEP = 16


def build_module():
    nc = bacc.Bacc("TRN2", target_bir_lowering=False, debug=False)

    fq_d = nc.dram_tensor("feature_q", [C, H, W], BF16, kind="ExternalInput").ap()
    fk_d = nc.dram_tensor("feature_k", [C, H, W], BF16, kind="ExternalInput").ap()
    fv_d = nc.dram_tensor("feature_v", [C, H, W], BF16, kind="ExternalInput").ap()
    wq_d = nc.dram_tensor("WqT", [C, M], BF16, kind="ExternalInput").ap()
    wk_d = nc.dram_tensor("WkT", [C, M], BF16, kind="ExternalInput").ap()
    wv_d = nc.dram_tensor("WvT", [C, C], BF16, kind="ExternalInput").ap()
    r_d = nc.dram_tensor("Rmat", [C, C], BF16, kind="ExternalInput").ap()
    out_d = nc.dram_tensor("out", [C, H, W], BF16, kind="ExternalOutput").ap()

    with tile.TileContext(nc) as tc:
        with tc.tile_pool(name="const", bufs=1) as cpool, \
             tc.tile_pool(name="persist", bufs=1) as pp, \
             tc.tile_pool(name="ps", bufs=1, space="PSUM") as ps, \
             tc.tile_pool(name="wk", bufs=1) as wkp, \
             tc.tile_pool(name="poolB", bufs=1) as pb:
            # ---- constants ----
            wq_sb = cpool.tile([128, CB, M], BF16, name="wq_sb")
            nc.sync.dma_start(wq_sb[:], wq_d.rearrange("(b p) m -> p b m", p=128))
            wk_sb = cpool.tile([128, CB, M], BF16, name="wk_sb")
            nc.sync.dma_start(wk_sb[:], wk_d.rearrange("(b p) m -> p b m", p=128))
            wv_sb = cpool.tile([128, CB, C], BF16, name="wv_sb")
            nc.sync.dma_start(wv_sb[:], wv_d.rearrange("(b p) c -> p b c", p=128))
            r_sb = cpool.tile([128, CB, C], BF16, name="r_sb")
            nc.sync.dma_start(r_sb[:], r_d.rearrange("(b p) c -> p b c", p=128))
            ones16 = cpool.tile([128, 2, 128], FP8E4, name="ones16")
            nc.vector.memset(ones16, 16.0)
            bias_t = cpool.tile([128, 1], F32, name="bias_t")
            nc.vector.memset(bias_t, EXP_BIAS)

            # ---- persistent tensors ----
            q4 = pp.tile([128, N], BF16, name="q4")          # q replicated x4
            kh = pp.tile([128, JG, 128], BF16, name="kh")    # [32*t+m, jg, jf]
            vt = pp.tile([128, JB, C], FP8E4, name="vt")     # vT[j, c] per jb
            fv_sb = pp.tile([128, CB, H, W], BF16, name="fv_sb")

            # ---- streamed projection of one 16-raw-row chunk.
            # q_fetch (DMA + pooling dy-add) is emitted ~9 steps before
            # q_proj's matmuls so the PE FIFO never head-blocks on the
            # fq DMA chain.
            qd_q = {}

            def q_fetch(icn, gps_dy):
                qr = wkp.tile([128, CB, 16, W], BF16, tag="qr", bufs=2,
                              name="qr")
                for cb in range(CB):
                    nc.gpsimd.dma_start(
                        qr[:, cb],
                        fq_d[cb * 128:(cb + 1) * 128,
                             icn * 16:(icn + 1) * 16, :])
                qd = wkp.tile([128, CB, 8, W], BF16, tag="qd", bufs=2,
                              name="qd")
                for cb in range(CB):
                    src = qr[:, cb].rearrange("c (h dy) w -> c h dy w", dy=2)
                    eng = nc.gpsimd if gps_dy else nc.vector
                    eng.tensor_add(qd[:, cb], src[:, :, 0], src[:, :, 1])
                qd_q[icn] = qd

            def q_proj(icn):
                qd = qd_q.pop(icn)
                pr = ps.tile([32, 512], F32, tag="u", bufs=1, name="prq")
                i = 0
                for cb in range(CB):
                    qdx = qd[:, cb].rearrange("c h (w dx) -> c h w dx", dx=2)
                    for dx in range(2):
                        nc.tensor.matmul(pr[:], wq_sb[:, cb],
                                         qdx[:, :, :, dx],
                                         start=(i == 0), stop=(i == 3),
                                         skip_group_check=True)
                        i += 1
                cs = slice(icn * 512, (icn + 1) * 512)
                nc.vector.tensor_copy(q4[0:32, cs], pr[:])
                for g in range(1, 4):
                    nc.sync.dma_start(q4[g * 32:(g + 1) * 32, cs],
                                      q4[0:32, cs])

            def q_pipeline(icn, gps_dy):
                q_fetch(icn, gps_dy)
                q_proj(icn)

            def kv_chunk(c):
                # fv chunk: raw rows into persistent fv_sb (residual+proj)
                for cb in range(CB):
                    nc.gpsimd.dma_start(
                        fv_sb[:, cb, c * 16:(c + 1) * 16, :],
                        fv_d[cb * 128:(cb + 1) * 128,
                             c * 16:(c + 1) * 16, :])
                # fk chunk: raw rows, transient
                fkr = wkp.tile([128, CB, 16, W], BF16, tag="fkr", bufs=3,
                               name="fkr")
                for cb in range(CB):
                    eng = nc.sync if cb == 0 else nc.scalar
                    eng.dma_start(
                        fkr[:, cb],
                        fk_d[cb * 128:(cb + 1) * 128,
                             c * 16:(c + 1) * 16, :])
                # dy-adds (contiguous, DVE)
                kd = wkp.tile([128, CB, 8, W], BF16, tag="kd", bufs=2,
                              name="kd")
                for cb in range(CB):
                    src = fkr[:, cb].rearrange("c (h dy) w -> c h dy w", dy=2)
                    nc.vector.tensor_add(kd[:, cb], src[:, :, 0],
                                         src[:, :, 1])
                vd = wkp.tile([128, CB, 8, W], BF16, tag="vd", bufs=2,
                              name="vd")
                for cb in range(CB):
                    src = fv_sb[:, cb, c * 16:(c + 1) * 16, :].rearrange(
                        "c (h dy) w -> c h dy w", dy=2)
                    nc.vector.tensor_add(vd[:, cb], src[:, :, 0],
                                         src[:, :, 1])
                # k-proj: dx pairs folded as accumulating matmuls
                pr = ps.tile([32, 512], F32, tag="u", bufs=1, name="prk")
                i = 0
                for cb in range(CB):
                    kdx = kd[:, cb].rearrange("c h (w dx) -> c h w dx", dx=2)
                    for dx in range(2):
                        nc.tensor.matmul(pr[:], wk_sb[:, cb],
                                         kdx[:, :, :, dx],
                                         start=(i == 0), stop=(i == 3),
                                         skip_group_check=True)
                        i += 1
                for t in range(4):
                    nc.vector.tensor_copy(kh[t * 32:(t + 1) * 32, c, :],
                                          pr[:, t * 128:(t + 1) * 128])
                # v-proj: per j-block (4 per chunk)
                for r in range(4):
                    jb = 4 * c + r
                    vt_ps = ps.tile([128, 512], F32, tag="u", bufs=1,
                                    name="vt_ps")[:, :C]
                    i = 0
                    for cb in range(CB):
                        vdx = vd[:, cb, 2 * r:2 * r + 2, :].rearrange(
                            "c h (w dx) -> c h w dx", dx=2)
                        for dx in range(2):
                            nc.tensor.matmul(vt_ps,
                                             vdx[:, :, :, dx],
                                             wv_sb[:, cb],
                                             start=(i == 0), stop=(i == 3),
                                             skip_group_check=True)
                            i += 1
                    nc.scalar.copy(vt[:, jb, :], vt_ps[:])

            # ---- phase B machinery ----
            def emit_S(ic, st, s_tile):
                i0 = ic * 512
                jgrp = st // 2
                for u in range(2):
                    t = 2 * (st % 2) + u
                    gs = slice(t * 32, (t + 1) * 32)
                    nc.tensor.matmul(s_tile[:, u, :], kh[gs, jgrp, :],
                                     q4[gs, i0:i0 + 512],
                                     start=True, stop=True,
                                     tile_position=(t * 32, 0),
                                     skip_group_check=True)

            def new_s():
                return ps.tile([128, 2, 512], F32, tag="s", bufs=2,
                               name="s_t")

            def epilogue_pe(ic, oc):
                # un-rotate: out2[c, i] = sum_c' R[c', c] oc[c', i]
                for cb in range(CB):
                    u_ps = ps.tile([128, 512], F32, tag="u", bufs=1,
                                   name="u_ps")
                    for cbp in range(CB):
                        nc.tensor.matmul(
                            u_ps[:], r_sb[:, cbp, cb * 128:(cb + 1) * 128],
                            oc[cbp][:],
                            start=(cbp == 0), stop=(cbp == CB - 1),
                            skip_group_check=True)
                    out3 = pb.tile([128, 512], BF16, tag="out3", bufs=2,
                                   name="out3")
                    nc.vector.tensor_copy(out3[:], u_ps[:])
                    final = pb.tile([128, 8, 2, WP, 2], BF16,
                                    tag="final", bufs=2, name="final")
                    up = out3.rearrange("c (h w) -> c h w", w=WP)[
                        :, :, :, None].to_broadcast((128, 8, WP, 2))
                    fvv = fv_sb[:, cb, ic * 16:(ic + 1) * 16, :].rearrange(
                        "c (h dy) (w dx) -> c h dy w dx", dy=2, dx=2)
                    nc.vector.tensor_add(final[:, :, 0], up, fvv[:, :, 0])
                    nc.gpsimd.tensor_add(final[:, :, 1], up, fvv[:, :, 1])
                    nc.sync.dma_start(
                        out_d[cb * 128:(cb + 1) * 128,
                              ic * 16:(ic + 1) * 16, :],
                        final.rearrange("c h dy w dx -> c (h dy) (w dx)"))

            # ---- interleaved schedule ----
            state = {"o": None, "l": None}
            s_q = []
            pending = []

            def begin_ic(ic):
                state["o"] = [ps.tile([128, 512], F32, tag=f"o{cb}",
                                      bufs=1, name=f"o{cb}_ps")
                              for cb in range(CB)]
                state["l"] = ps.tile([128, 512], F32, tag="l", bufs=1,
                                     name="l_ps")

            def run_step(ic, st):
                s_cur = s_q.pop(0)
                nxt = (ic, st + 2) if st + 2 < NSTEP else (
                    (ic + 1, st - 14) if ic + 1 < IC else None)
                if nxt is not None:
                    s_new = new_s()
                    emit_S(nxt[0], nxt[1], s_new)
                    s_q.append(s_new)
                p = pb.tile([128, 2, 512], FP8E5, tag="p", bufs=4, name="p")
                nc.scalar.activation(p[:], s_cur[:], AF.Exp,
                                     scale=0.0625, bias=bias_t[:])
                nc.tensor.matmul(
                    state["l"][:], ones16[:], p[:],
                    start=(st == 0), stop=(st == NSTEP - 1),
                    perf_mode=DR, skip_group_check=True)
                for cb in range(CB):
                    nc.tensor.matmul(
                        state["o"][cb][:],
                        vt[:, 2 * st:2 * st + 2, cb * 128:(cb + 1) * 128],
                        p[:],
                        start=(st == 0), stop=(st == NSTEP - 1),
                        perf_mode=DR, skip_group_check=True)
                if st == 5 and pending:
                    pending.pop(0)()
                if st == 2 and ic + 1 < IC:
                    q_proj(ic + 1)
                if st == 11 and ic + 2 < IC:
                    q_fetch(ic + 2, gps_dy=True)

            def end_ic(ic):
                # epilogue head: free PSUM banks fast
                rb = pb.tile([128, 512], F32, tag="rb", bufs=2, name="rb")
                nc.vector.reciprocal_approx_fast(out=rb[:],
                                                 in_=state["l"][:])
                oc = []
                for cb in range(CB):
                    osb = pb.tile([128, 512], BF16, tag=f"osb{cb}", bufs=2,
                                  name=f"osb{cb}")
                    nc.vector.tensor_copy(osb[:], state["o"][cb][:])
                    t_ = pb.tile([128, 512], BF16, tag=f"oc{cb}", bufs=2,
                                 name=f"oc{cb}")
                    nc.vector.tensor_mul(t_[:], osb[:], rb[:])
                    oc.append(t_)
                pending.append(lambda ic_=ic, oc_=oc: epilogue_pe(ic_, oc_))

            # phase A': stream chunks; interleave ic0 steps with 1-chunk lag
            q_pipeline(0, gps_dy=False)
            begin_ic(0)
            for c in range(IC):
                kv_chunk(c)
                if c == 1:
                    q_fetch(1, gps_dy=True)
                if c == 0:
                    s_q.append(new_s())
                    emit_S(0, 0, s_q[0])
                    s_q.append(new_s())
                    emit_S(0, 1, s_q[1])
                else:
                    run_step(0, 2 * (c - 1))
                    run_step(0, 2 * (c - 1) + 1)
            run_step(0, 14)
            run_step(0, 15)
            end_ic(0)
            # steady state: ic 1..7
            for ic in range(1, IC):
                begin_ic(ic)
                for st in range(NSTEP):
                    run_step(ic, st)
                end_ic(ic)
            for fn in pending:
                fn()

    nc.compile()
    return nc


def _hadamard(n):
    h = np.array([[1.0]], dtype=np.float64)
    while h.shape[0] < n:
        h = np.block([[h, h], [h, -h]])
    return h / np.sqrt(n)


_NC_CACHE = []
LAST_RESULT = []  # last BassKernelResults, for perf inspection by test.py


def kernel(**inputs) -> np.ndarray:
    bf = ml_dtypes.bfloat16
    fq = np.ascontiguousarray(np.asarray(inputs["feature_q"]).astype(bf))
    fk = np.ascontiguousarray(np.asarray(inputs["feature_k"]).astype(bf))
    fv = np.ascontiguousarray(np.asarray(inputs["feature_v"]).astype(bf))
    wq = np.asarray(inputs["Wq"], dtype=np.float32)
    wk = np.asarray(inputs["Wk"], dtype=np.float32)
    wv = np.asarray(inputs["Wv"], dtype=np.float32)

    # on-device pooling is a 2x2 *sum*; q,k each pick up 4x -> s is 16x,
    # folded into the exp scale. v picks up 4x from pooling; another 4x
    # here scales v into fp8e4 range (16x total, cancelled by the
    # 16.0-weighted denominator). Wv is pre-rotated by R (Hadamard/16,
    # orthonormal, symmetric, bf16-exact); the kernel un-rotates after
    # the attention average.
    R = _hadamard(256)
    wqt = np.ascontiguousarray(wq.T.astype(bf))               # (C, M)
    wkt = np.ascontiguousarray(wk.T.astype(bf))
    wvt = np.ascontiguousarray(((R @ wv).T * 4.0).astype(bf))  # (C, C)
    rmat = np.ascontiguousarray(R.astype(bf))                  # (C, C)

    if not _NC_CACHE:
        _NC_CACHE.append(build_module())
    nc = _NC_CACHE[0]

    in_maps = [
        {
            "feature_q": fq[b],
            "feature_k": fk[b],
            "feature_v": fv[b],
            "WqT": wqt,
            "WkT": wkt,
            "WvT": wvt,
            "Rmat": rmat,
        }
        for b in range(B)
    ]
    res = run_bass_kernel_spmd(nc, in_maps, core_ids=list(range(B)))
    LAST_RESULT.clear()
    LAST_RESULT.append(res)
    out = np.stack([res.results[b]["out"] for b in range(B)], axis=0)
    return out.astype(np.float32)


if __name__ == "__main__":
    nc = build_module()
    print("module built + compiled OK")
